# revision 1
# baseline (speedup 1.0000x reference)
"""Sparse 3D conv (gather -> per-offset GEMM -> scatter-add over K) on 8 trn2 cores.

Computation: out[m, o] = sum_k sum_c feats[in_idx[k, m], c] * mask[k, m] * kernel[k, c, o]

Strategy (per core, output voxels sharded 8 ways):
  - Host: feats -> fp16 with an appended all-zero sentinel row; masked (k, m)
    pairs point at the sentinel, so no mask multiply is needed on device.
    27 offsets padded to 28 = 7 groups of 4; 4 offsets x 32 channels = 128
    = one full contraction. Indices pre-arranged so that a single SWDGE
    indirect gather + one DVE 32x32 stream-transpose yields tiles with the
    (k-slot, channel) contraction dim on partitions.
  - Device: for each 512-voxel tile: indirect-gather 14336 fp16 rows,
    stream-transpose, 7 accumulating fp16 matmuls into PSUM [64, 512] f32,
    copy to SBUF, DMA to the output laid out [64, M] (transposed on host).
"""

import numpy as np

N_VOX = 200000
M_VOX = 100000
K_VOL = 27
C_IN = 32
C_OUT = 64
N_CORES = 8
M_CORE = M_VOX // N_CORES        # 12500
TILE_M = 512
N_TILES_FULL = (M_CORE + TILE_M - 1) // TILE_M  # 25
M_PAD = N_TILES_FULL * TILE_M    # 12800
N_G = 7                          # groups of 4 k-offsets (28 slots, last padded)
JJ = N_G * 16                    # 112 32-column j-blocks per 512-voxel tile
SENT = N_VOX                     # index of the appended zero row


def _build_program(n_tiles=N_TILES_FULL):
    import concourse.tile as tile
    import concourse.mybir as mybir
    from concourse import bacc
    from concourse.bass import IndirectOffsetOnAxis
    from concourse._compat import get_trn_type

    nc = bacc.Bacc(get_trn_type() or "TRN2", target_bir_lowering=False, debug=False,
                   num_swdge_queues=4)

    feats_h = nc.dram_tensor(
        "feats16", (N_VOX + 1, C_IN), mybir.dt.float16, kind="ExternalInput")
    idx_h = nc.dram_tensor(
        "idx_sb", (128, n_tiles * JJ), mybir.dt.int32, kind="ExternalInput")
    w_h = nc.dram_tensor(
        "w_sb", (128, N_G * C_OUT), mybir.dt.float16, kind="ExternalInput")
    out_h = nc.dram_tensor(
        "out_t", (C_OUT, n_tiles * TILE_M), mybir.dt.float32, kind="ExternalOutput")

    with tile.TileContext(nc) as tc:
        with (
            tc.tile_pool(name="const", bufs=1) as const,
            tc.tile_pool(name="gath", bufs=3) as gath,
            tc.tile_pool(name="trans", bufs=3) as trans,
            tc.tile_pool(name="psum", bufs=4, space="PSUM") as psum,
            tc.tile_pool(name="outp", bufs=3) as outp,
        ):
            idx_sb = const.tile([128, n_tiles * JJ], mybir.dt.int32)
            nc.sync.dma_start(idx_sb[:], idx_h[:])
            w_sb = const.tile([128, N_G * C_OUT], mybir.dt.float16)
            nc.sync.dma_start(w_sb[:], w_h[:])

            for t in range(n_tiles):
                g_t = gath.tile([128, JJ * C_IN], mybir.dt.float16, tag="g")
                for jj in range(JJ):
                    # one offset per partition, contiguous [128, 32] dest — the
                    # only indirect-DMA shape walrus lowers correctly
                    gi = nc.gpsimd.indirect_dma_start(
                        out=g_t[:, jj * C_IN:(jj + 1) * C_IN],
                        out_offset=None,
                        in_=feats_h[:],
                        in_offset=IndirectOffsetOnAxis(
                            ap=idx_sb[:, t * JJ + jj:t * JJ + jj + 1], axis=0),
                    )
                    # spread DGE work across the 4 SWDGE queues (Q7 core pairs)
                    q = jj % 4
                    if q:
                        gi.ins.queue = f"qPoolDynamic{q}"
                t_t = trans.tile([128, JJ * C_IN], mybir.dt.float16, tag="t")
                nc.vector.transpose(t_t[:], g_t[:])
                ps = psum.tile([C_OUT, TILE_M], mybir.dt.float32, tag="ps")
                for g in range(N_G):
                    nc.tensor.matmul(
                        out=ps[:],
                        lhsT=w_sb[:, g * C_OUT:(g + 1) * C_OUT],
                        rhs=t_t[:, g * TILE_M:(g + 1) * TILE_M],
                        start=(g == 0),
                        stop=(g == N_G - 1),
                    )
                ob = outp.tile([C_OUT, TILE_M], mybir.dt.float32, tag="ob")
                nc.scalar.copy(ob[:], ps[:])
                nc.sync.dma_start(out_h[:, t * TILE_M:(t + 1) * TILE_M], ob[:])

    nc.compile()
    return nc


def pack_inputs(feats, kernel, in_idx, mask):
    """Host-side packing. Returns (feats16, per-core idx arrays, w_sb)."""
    feats = np.asarray(feats, np.float32)
    kernel = np.asarray(kernel, np.float32)
    in_idx = np.asarray(in_idx)
    mask = np.asarray(mask)

    feats16 = np.concatenate(
        [feats, np.zeros((1, C_IN), np.float32)], 0).astype(np.float16)
    eidx = np.where(mask != 0, in_idx, SENT).astype(np.int32)  # [27, M]

    idx_cores = []
    for core in range(N_CORES):
        sl = eidx[:, core * M_CORE:(core + 1) * M_CORE]
        full = np.full((N_G * 4, M_PAD), SENT, np.int32)
        full[:K_VOL, :M_CORE] = sl
        # idx_sb[32*kk+pp, (t*N_G+g)*16+j] = full[4g+kk, t*512+32j+pp]
        a = full.reshape(N_G, 4, N_TILES_FULL, 16, 32)   # [g, kk, t, j, pp]
        idx_sb = np.transpose(a, (1, 4, 2, 0, 3)).reshape(128, N_TILES_FULL * JJ)
        idx_cores.append(np.ascontiguousarray(idx_sb))

    kpad = np.zeros((N_G * 4, C_IN, C_OUT), np.float32)
    kpad[:K_VOL] = kernel
    w_sb = np.transpose(
        kpad.reshape(N_G, 4, C_IN, C_OUT), (1, 2, 0, 3)).reshape(128, N_G * C_OUT)
    return feats16, idx_cores, np.ascontiguousarray(w_sb.astype(np.float16))


_NC_CACHE = {}


def get_program(n_tiles=N_TILES_FULL):
    if n_tiles not in _NC_CACHE:
        _NC_CACHE[n_tiles] = _build_program(n_tiles)
    return _NC_CACHE[n_tiles]


def run_on_device(feats16, idx_cores, w_sb, trace=False, tmpdir=None):
    from concourse import bass_utils
    from concourse.bass_interp import get_hw_module

    nc = get_program()
    in_maps = [
        {"feats16": feats16, "idx_sb": idx_cores[c], "w_sb": w_sb}
        for c in range(N_CORES)
    ]
    old_m = nc.m
    nc.m = get_hw_module(nc.m)
    try:
        res = bass_utils.run_bass_kernel_spmd(
            nc, in_maps, core_ids=list(range(N_CORES)), trace=trace,
            tmpdir=tmpdir)
    finally:
        nc.m = old_m
    return res


def kernel(feats, kernel, in_idx, mask):
    feats16, idx_cores, w_sb = pack_inputs(feats, kernel, in_idx, mask)
    res = run_on_device(feats16, idx_cores, w_sb)
    outs = [res.results[c]["out_t"][:, :M_CORE].T for c in range(N_CORES)]
    return np.ascontiguousarray(np.concatenate(outs, 0), dtype=np.float32)

